# revision 1
# baseline (speedup 1.0000x reference)
"""GraphGym GeneralConv (GCN-style, add-aggr, symmetric norm) on 8 Trainium2
NeuronCores via Bass/Tile.

Math (matches the reference exactly, up to fp reassociation):
    deg[i]  = 1 + #{e : row[e] == i}
    dis     = deg ** -0.5
    h       = x @ W
    out[i]  = dis[i] * ( sum_{e : col[e] == i} dis[row[e]] * h[row[e]]
                         + dis[i] * h[i] )          # self-loop

Distribution: destination-node sharding.  Core k owns dest nodes
[k*SHARD, (k+1)*SHARD); every edge is routed to the core owning its dest.
Every core redundantly computes the full h = x @ W (x is replicated,
transposed on the host so no on-device transpose is needed), writes it to a
DRAM scratch `h_perm` in a permuted tile-major row layout (so the write is
a single contiguous DMA per slice), then gathers its edges' source rows
edge-major with the SWDGE dma_gather instruction (int16 indices wrapped
over 16 partitions, source windowed into 4 chunks of <=32767 rows),
scatter-adds them into per-dest-block PSUM accumulators with
selection-matrix matmuls on the PE (sel[p, d] = (col_local[slot p] == d) *
dis[row[slot p]], built by one fused DVE tensor_scalar per 128-slot tile),
adds the self-loop term with a diagonal matmul over indirectly-gathered own
rows, scales by dis[dest], and writes its shard as one contiguous DMA.

The host does integer-only preprocessing: degree histogram, edge bucketing
by (core, dest-block, source-chunk), fixed-size slot layout, index/col
packing.  The slot layout is input-independent (fixed run length L_RUN per
bucket), so the Bass program is compiled once and cached; bucket overflow
(a few hundred edges for random graphs) is corrected exactly on the host.
"""

import math

import numpy as np

# ----------------------------------------------------------------------------
# configuration
# ----------------------------------------------------------------------------

N_NODES = 100000
DIM = 64
N_CORES = 8

P = 128  # partitions


class Cfg:
    def __init__(self, n_nodes, dim, n_cores, slots_per_run,
                 blocks_per_group, chunk_slices=2, subcall_runs=None):
        self.N = n_nodes
        self.DIM = dim
        self.NC = n_cores
        # 128-aligned dest shards; the last core's shard may be smaller
        self.NBLK = math.ceil(n_nodes / (n_cores * P))   # dest blocks / core
        self.SHARD = self.NBLK * P               # padded shard size
        # h-permutation slice == shard, uniform J so the per-core self-row
        # window is one affine AP at a register offset
        self.SLICE = self.SHARD
        self.NS = n_cores
        self.J = [self.NBLK] * n_cores
        self.row_base = np.arange(n_cores + 1) * self.SHARD
        self.H_ROWS = int(self.row_base[-1])
        assert self.SHARD * (n_cores - 1) < n_nodes <= self.H_ROWS
        self.CH_SL = chunk_slices                # slices per gather chunk
        self.NCH = math.ceil(self.NS / chunk_slices)
        self.crow = [int(self.row_base[min(c * chunk_slices, self.NS)])
                     for c in range(self.NCH + 1)]
        for c in range(self.NCH):
            assert self.crow[c + 1] - self.crow[c] <= 32767
        self.L_RUN = int(slots_per_run)          # slots per (block,chunk) run
        assert self.L_RUN % 64 == 0
        self.NBG = blocks_per_group              # dest blocks per psum group
        assert self.NBLK % blocks_per_group == 0
        self.NGRP = self.NBLK // blocks_per_group
        self.TOT = self.NBLK * self.NCH * self.L_RUN   # slots per core
        assert self.TOT % P == 0
        self.NTILES = self.TOT // P
        self.SR = subcall_runs or blocks_per_group  # runs per dma_gather call
        assert self.NBG % self.SR == 0
        assert (self.SR * self.L_RUN) % P == 0
        self.CALL_SLOTS = self.SR * self.L_RUN
        self.NCALLS = self.NGRP * self.NCH * (self.NBG // self.SR)
        self.IDXW = self.TOT // 16

    def run_subtiles(self, g, c, b_):
        """K-subtiles of run (g, c, b_): [(abs_slot0, K), ...]."""
        out = []
        base = ((g * self.NCH + c) * self.NBG + b_) * self.L_RUN
        s = base
        end = s + self.L_RUN
        while s < end:
            if s % P:
                k = P - s % P
            else:
                k = min(P, end - s)
            out.append((s, k))
            s += k
        return out


CFG = Cfg(N_NODES, DIM, N_CORES, slots_per_run=640,
          blocks_per_group=7, subcall_runs=7)


def rho(cfg, n):
    """node id -> permuted h_perm row (vectorized)."""
    s = n // cfg.SLICE
    m = n - s * cfg.SLICE
    J = np.asarray(cfg.J)[s]
    return cfg.row_base[s] + (m % P) * J + m // P


# ----------------------------------------------------------------------------
# host preprocessing
# ----------------------------------------------------------------------------

def host_prep(cfg, x, weight, edge_index):
    x = np.asarray(x, dtype=np.float32)
    weight = np.asarray(weight, dtype=np.float32)
    ei = np.asarray(edge_index)
    row = ei[0].astype(np.int64)
    col = ei[1].astype(np.int64)

    # deg counts outgoing (row) edges plus the implicit self-loop
    deg = (np.bincount(row, minlength=cfg.N) + 1).astype(np.float32)

    k = np.minimum(col // cfg.SHARD, cfg.NC - 1)
    blk = (col % cfg.SHARD) // P
    col_local = (col % cfg.SHARD) % P
    g = blk // cfg.NBG
    b_ = blk % cfg.NBG
    s = row // cfg.SLICE
    c = np.minimum(s // cfg.CH_SL, cfg.NCH - 1)
    prow = rho(cfg, row)
    idxrel = prow - np.asarray(cfg.crow)[c]

    run_in_core = (g * cfg.NCH + c) * cfg.NBG + b_
    key = k * (cfg.NBLK * cfg.NCH) + run_in_core
    order = np.argsort(key, kind="stable")
    key_s = key[order]
    counts = np.bincount(key_s, minlength=cfg.NC * cfg.NBLK * cfg.NCH)
    starts = np.concatenate([[0], np.cumsum(counts)])
    pos = np.arange(key_s.size) - starts[key_s]

    ok = pos < cfg.L_RUN
    slot = run_in_core[order] * cfg.L_RUN + pos   # slot within core
    kk = k[order]

    idx_flat = np.zeros((cfg.NC, cfg.TOT), dtype=np.int16)
    colv = np.full((cfg.NC, cfg.TOT), -1.0, dtype=np.float32)
    degrow = np.ones((cfg.NC, cfg.TOT), dtype=np.float32)

    o = order[ok]
    idx_flat[kk[ok], slot[ok]] = idxrel[o].astype(np.int16)
    colv[kk[ok], slot[ok]] = col_local[o].astype(np.float32)
    degrow[kk[ok], slot[ok]] = deg[row[o]]

    # overflow edges -> host correction (expected: a handful)
    ov = order[~ok]

    def pack(a):
        return np.ascontiguousarray(
            a.reshape(cfg.NC, cfg.NTILES, P).transpose(0, 2, 1))

    # per-call 16-partition wrapping of indices, replicated to 128 partitions
    idxw = idx_flat.reshape(cfg.NC, cfg.NCALLS, cfg.CALL_SLOTS // 16, 16)
    idxw = idxw.transpose(0, 3, 1, 2).reshape(cfg.NC, 16, cfg.IDXW)
    idxv_p = np.ascontiguousarray(np.tile(idxw, (1, 8, 1)))

    colv_p = pack(colv)
    degrow_p = pack(degrow)

    # dest-side degrees [NC, 128, NBLK]
    degdest = np.ones((cfg.NC, cfg.NBLK * P), dtype=np.float32)
    ids = np.arange(cfg.SHARD)
    for core in range(cfg.NC):
        nd = min(cfg.SHARD, cfg.N - core * cfg.SHARD)
        degdest[core, :nd] = deg[core * cfg.SHARD + ids[:nd]]
    degdest = np.ascontiguousarray(
        degdest.reshape(cfg.NC, cfg.NBLK, P).transpose(0, 2, 1))

    # per-block self rows: permuted h row of each dest node [NC, 128, NBLK]
    selfidx = np.zeros((cfg.NC, cfg.NBLK * P), dtype=np.int32)
    for core in range(cfg.NC):
        nd = min(cfg.SHARD, cfg.N - core * cfg.SHARD)
        selfidx[core, :nd] = rho(cfg, core * cfg.SHARD + ids[:nd])
    selfidx = np.ascontiguousarray(
        selfidx.reshape(cfg.NC, cfg.NBLK, P).transpose(0, 2, 1))

    xt = np.ascontiguousarray(x.T)
    iota = np.broadcast_to(np.arange(P, dtype=np.float32), (P, P)).copy()
    partidx = np.arange(P, dtype=np.float32).reshape(P, 1).copy()

    in_maps = []
    for core in range(cfg.NC):
        in_maps.append({
            "xt": xt,
            "w": weight,
            "iota": iota,
            "partidx": partidx,
            "colv": colv_p[core],
            "degrow": degrow_p[core],
            "degdest": degdest[core],
            "selfidx": selfidx[core],
            "idx": idxv_p[core],
        })

    # host correction for overflowed edges
    corr = None
    if ov.size:
        r, cdst = row[ov], col[ov]
        hsrc = x[r] @ weight
        m = hsrc * (deg[r] ** -0.5 * deg[cdst] ** -0.5)[:, None]
        corr = np.zeros((cfg.N, cfg.DIM), dtype=np.float32)
        np.add.at(corr, cdst, m)
    return in_maps, corr


def unshard(cfg, outs, corr):
    out = np.empty((cfg.N, cfg.DIM), dtype=np.float32)
    for core in range(cfg.NC):
        o = outs[core]["outp"].reshape(P, cfg.NBLK, cfg.DIM)
        o = o.transpose(1, 0, 2).reshape(cfg.NBLK * P, cfg.DIM)
        nd = min(cfg.SHARD, cfg.N - core * cfg.SHARD)
        out[core * cfg.SHARD:core * cfg.SHARD + nd] = o[:nd]
    if corr is not None:
        out += corr
    return out


# ----------------------------------------------------------------------------
# device program
# ----------------------------------------------------------------------------

_PROG_CACHE = {}


def build_program(cfg, reps=1, phases="12", queue_map=None):
    import contextlib

    import concourse.bass as bass
    import concourse.tile as tile
    from concourse import bacc, mybir

    f32 = mybir.dt.float32
    nc = bacc.Bacc("TRN2", target_bir_lowering=False, debug=False,
                   num_devices=cfg.NC, num_swdge_queues=4)

    xt = nc.dram_tensor("xt", [cfg.DIM, cfg.N], f32, kind="ExternalInput")
    w = nc.dram_tensor("w", [cfg.DIM, cfg.DIM], f32, kind="ExternalInput")
    iota = nc.dram_tensor("iota", [P, P], f32, kind="ExternalInput")
    partidx = nc.dram_tensor("partidx", [P, 1], f32, kind="ExternalInput")
    selfidx = nc.dram_tensor("selfidx", [P, cfg.NBLK], mybir.dt.int32,
                             kind="ExternalInput")
    colv = nc.dram_tensor("colv", [P, cfg.NTILES], f32, kind="ExternalInput")
    degrow = nc.dram_tensor("degrow", [P, cfg.NTILES], f32,
                            kind="ExternalInput")
    degdest = nc.dram_tensor("degdest", [P, cfg.NBLK], f32,
                             kind="ExternalInput")
    idx = nc.dram_tensor("idx", [P, cfg.IDXW], mybir.dt.int16,
                         kind="ExternalInput")
    outp = nc.dram_tensor("outp", [P, cfg.NBLK * cfg.DIM], f32,
                          kind="ExternalOutput")
    h_perm = nc.dram_tensor("h_perm", [cfg.H_ROWS, cfg.DIM], f32)

    PSB = 8  # h tiles batched per psum bank
    nc._gather_insts = []

    with tile.TileContext(nc) as tc:
      with (tc.For_i(0, reps, 1) if reps > 1 else contextlib.nullcontext()):
        # ---------------- phase 1: h = x @ W, permuted layout ----------------
        if "1" in phases:
            with tc.tile_pool(name="p1s", bufs=2) as sp, \
                 tc.tile_pool(name="p1c", bufs=1) as cp, \
                 tc.tile_pool(name="p1p", bufs=4, space="PSUM") as pp:
                w_sb = cp.tile([cfg.DIM, cfg.DIM], f32)
                nc.sync.dma_start(out=w_sb[:], in_=w[:])
                for s in range(cfg.NS):
                    J = cfg.J[s]
                    n0 = s * cfg.SLICE
                    nn = min(cfg.SLICE, cfg.N - n0)
                    xs = sp.tile([cfg.DIM, cfg.SLICE], f32, tag="xs")
                    nc.sync.dma_start(out=xs[:, :nn], in_=xt[:, n0:n0 + nn])
                    if nn < P * J:
                        # pad the tail tile so every psum row is written
                        nc.vector.memset(xs[:, nn:P * J], 0)
                    hs = sp.tile([P, cfg.J[0] * cfg.DIM], f32, tag="hs")
                    for m in range(math.ceil(J / PSB)):
                        j0 = m * PSB
                        jn = min(PSB, J - j0)
                        ps = pp.tile([P, PSB * cfg.DIM], f32)
                        for j8 in range(jn):
                            j = j0 + j8
                            nc.tensor.matmul(
                                out=ps[:, j8 * cfg.DIM:(j8 + 1) * cfg.DIM],
                                lhsT=xs[:, j * P:(j + 1) * P],
                                rhs=w_sb[:],
                                start=True, stop=True)
                        nc.vector.tensor_copy(
                            out=hs[:, j0 * cfg.DIM:(j0 + jn) * cfg.DIM],
                            in_=ps[:, :jn * cfg.DIM])
                    dst = h_perm[cfg.row_base[s]:cfg.row_base[s] + P * J, :]
                    dst = dst.rearrange("(p j) d -> p (j d)", p=P)
                    nc.sync.dma_start(out=dst, in_=hs[:, :J * cfg.DIM])

        # ---------------- phase 2: indirect gather + PE scatter-add ---------
        if set("2GM") & set(phases):
            mode = ("full" if "2" in phases else
                    "gather" if "G" in phases else "mm")
            with tc.tile_pool(name="p2c", bufs=1) as cp, \
                 tc.tile_pool(name="p2g", bufs=3) as gp, \
                 tc.tile_pool(name="p2sel", bufs=6) as selp, \
                 tc.tile_pool(name="p2p", bufs=2, space="PSUM") as pp:
                iota_sb = cp.tile([P, P], f32)
                nc.sync.dma_start(out=iota_sb[:], in_=iota[:])
                pidx_sb = cp.tile([P, 1], f32)
                nc.sync.dma_start(out=pidx_sb[:], in_=partidx[:])
                colv_sb = cp.tile([P, cfg.NTILES], f32)
                nc.sync.dma_start(out=colv_sb[:], in_=colv[:])
                selv_sb = cp.tile([P, cfg.NTILES], f32)
                nc.sync.dma_start(out=selv_sb[:], in_=degrow[:])
                # dis[row] = 1/sqrt(deg[row])
                nc.scalar.sqrt(out=selv_sb[:], in_=selv_sb[:])
                nc.vector.reciprocal(out=selv_sb[:], in_=selv_sb[:])
                disd_sb = cp.tile([P, cfg.NBLK], f32)
                nc.sync.dma_start(out=disd_sb[:], in_=degdest[:])
                nc.scalar.sqrt(out=disd_sb[:], in_=disd_sb[:])
                nc.vector.reciprocal(out=disd_sb[:], in_=disd_sb[:])
                sidx_sb = cp.tile([P, cfg.NBLK], mybir.dt.int32)
                nc.sync.dma_start(out=sidx_sb[:], in_=selfidx[:])
                idx_sb = cp.tile([P, cfg.IDXW], mybir.dt.int16)
                nc.sync.dma_start(out=idx_sb[:], in_=idx[:])
                out_sb = cp.tile([P, cfg.NBLK * cfg.DIM], f32)

                if mode == "mm":
                    shared_sel = cp.tile([P, P], f32)
                    nc.vector.tensor_scalar(
                        out=shared_sel[:], in0=iota_sb[:],
                        scalar1=colv_sb[:, 0:1], scalar2=selv_sb[:, 0:1],
                        op0=mybir.AluOpType.is_equal,
                        op1=mybir.AluOpType.mult)

                gbufs = {}
                CW = cfg.CALL_SLOTS // 16   # idx columns per call
                CT = cfg.CALL_SLOTS // P    # slot tiles per call

                def get_gbuf(T):
                    j = T // CT
                    if j not in gbufs:
                        c = (j // (cfg.NBG // cfg.SR)) % cfg.NCH
                        gb = gp.tile([P, CT, cfg.DIM], f32, tag="gbuf")
                        gi = nc.gpsimd.dma_gather(
                            out_ap=gb[:],
                            in_ap=h_perm[cfg.crow[c]:cfg.crow[c + 1], :],
                            idxs_ap=idx_sb[:, j * CW:(j + 1) * CW],
                            num_idxs=cfg.CALL_SLOTS,
                            num_idxs_reg=cfg.CALL_SLOTS,
                            elem_size=cfg.DIM,
                            single_packet=False,
                            queue_num=(queue_map or {}).get(j, 0),
                        )
                        nc._gather_insts.append((j, gi.ins.name))
                        gbufs[j] = gb
                        if mode == "gather":
                            nc.vector.tensor_copy(out=out_sb[:, :cfg.DIM],
                                                  in_=gb[:, 0, :])
                    return gbufs[j], T % CT

                bank_w = 2048 // (4 * cfg.DIM)  # blocks per psum bank
                for g in range(cfg.NGRP):
                    if mode == "gather":
                        for c in range(cfg.NCH):
                            for b_ in range(cfg.NBG):
                                for (s0, kk) in cfg.run_subtiles(g, c, b_):
                                    get_gbuf(s0 // P)
                        continue
                    ps = pp.tile([P, cfg.NBG * cfg.DIM], f32)
                    for c in range(cfg.NCH):
                      for b_ in range(cfg.NBG):
                        for si, (s0, kk) in enumerate(cfg.run_subtiles(g, c, b_)):
                            gb, tloc = get_gbuf(s0 // P)
                            T = s0 // P
                            p0 = s0 % P
                            if mode == "mm":
                                sel = shared_sel
                            else:
                                sel = selp.tile([P, P], f32)
                                nc.vector.tensor_scalar(
                                    out=sel[p0:p0 + kk, :],
                                    in0=iota_sb[p0:p0 + kk, :],
                                    scalar1=colv_sb[p0:p0 + kk, T:T + 1],
                                    scalar2=selv_sb[p0:p0 + kk, T:T + 1],
                                    op0=mybir.AluOpType.is_equal,
                                    op1=mybir.AluOpType.mult)
                            nc.tensor.matmul(
                                out=ps[:, b_ * cfg.DIM:(b_ + 1) * cfg.DIM],
                                lhsT=sel[p0:p0 + kk, :],
                                rhs=gb[p0:p0 + kk, tloc, :],
                                start=(b_ % bank_w == 0 and c == 0
                                       and si == 0),
                                stop=False, skip_group_check=True)
                    for b_ in range(cfg.NBG):
                        b = g * cfg.NBG + b_
                        # self-loop: psum[:, b_] += diag(dis[dest]) @ h[dest]
                        hself = selp.tile([P, cfg.DIM], f32, tag="hself")
                        nc.gpsimd.indirect_dma_start(
                            out=hself[:], out_offset=None,
                            in_=h_perm[:],
                            in_offset=bass.IndirectOffsetOnAxis(
                                ap=sidx_sb[:, b:b + 1], axis=0))
                        diag = selp.tile([P, P], f32, tag="diag")
                        nc.vector.tensor_scalar(
                            out=diag[:], in0=iota_sb[:],
                            scalar1=pidx_sb[:, 0:1],
                            scalar2=disd_sb[:, b:b + 1],
                            op0=mybir.AluOpType.is_equal,
                            op1=mybir.AluOpType.mult)
                        nc.tensor.matmul(
                            out=ps[:, b_ * cfg.DIM:(b_ + 1) * cfg.DIM],
                            lhsT=diag[:], rhs=hself[:],
                            start=False, stop=True, skip_group_check=True)
                        nc.vector.tensor_scalar_mul(
                            out_sb[:, b * cfg.DIM:(b + 1) * cfg.DIM],
                            ps[:, b_ * cfg.DIM:(b_ + 1) * cfg.DIM],
                            disd_sb[:, b:b + 1])
                nc.sync.dma_start(out=outp[:], in_=out_sb[:])

    nc.compile()
    return nc


def gather_queue_map(nc):
    """call_j -> queue: DMASW lane % 4, except lanes also used by plain
    Pool DMAs (which are implicitly queue 0) are pinned to queue 0."""
    import concourse.mybir as mybir
    from concourse.tile_sem_assignment import PROC_NAME_TO_IDX
    idx2name = {v: k for k, v in PROC_NAME_TO_IDX.items()}
    gather_names = {name for _, name in nc._gather_insts}
    locked = set()
    for name, inst in nc.inst_map.items():
        proc = idx2name.get(getattr(inst, "bass_scheduled_proc", None), "")
        if (proc.startswith("DMASW") and name not in gather_names):
            locked.add(proc)
    qm = {}
    for j, name in nc._gather_insts:
        inst = nc.inst_map[name]
        proc = idx2name[inst.bass_scheduled_proc]
        assert proc.startswith("DMASW")
        qm[j] = 0 if proc in locked else int(proc[5:]) % 4
    return qm


def build_with_queues(cfg, reps=1, phases="12", rotate=False):
    if not rotate:
        return build_program(cfg, reps=reps, phases=phases, queue_map=None)
    qm = {}
    for _ in range(3):
        nc = build_program(cfg, reps=reps, phases=phases, queue_map=qm)
        qm2 = gather_queue_map(nc)
        if qm2 == qm:
            return nc
        qm = qm2
    return build_program(cfg, reps=reps, phases=phases, queue_map=None)


def get_program(cfg):
    key = (cfg.N, cfg.DIM, cfg.NC, cfg.SLICE, cfg.L_RUN, cfg.NBG,
           cfg.CH_SL, cfg.SR)
    if key not in _PROG_CACHE:
        _PROG_CACHE[key] = build_with_queues(cfg)
    return _PROG_CACHE[key]


# ----------------------------------------------------------------------------
# entry point
# ----------------------------------------------------------------------------

def kernel(x, weight, edge_index):
    from concourse.bass_utils import run_bass_kernel_spmd

    cfg = CFG
    in_maps, corr = host_prep(cfg, x, weight, edge_index)
    nc = get_program(cfg)
    res = run_bass_kernel_spmd(nc, in_maps, list(range(cfg.NC)))
    return unshard(cfg, res.results, corr)



# revision 2
# speedup vs baseline: 1.1846x; 1.1846x over previous
"""GraphGym GeneralConv (GCN-style, add-aggr, symmetric norm) on 8 TRN2 cores.

v3 — v2 plus:
  * phase-1 matmuls and x inputs in bf16 (fp32 PE matmuls are ~8x slower)
  * h_perm split into 4 per-chunk DRAM tensors and gather calls ordered
    chunk-major, so chunk-c gathers only wait for slices 2c/2c+1 — phase 1
    overlaps under the gather pipe
  * per-call PSUM accumulated into an SBUF accumulator by DVE (seeded with
    the self term), final dis[dest] scale as one broadcast multiply
  * the self pass (xself @ W) runs before phase 1 so the seed is ready early

Math:
    deg[i] = 1 + outdeg[i];  dis = deg**-0.5
    hs[r]  = dis[r] * (x[r] @ W)           (stored permuted, bf16, 128-wide)
    out[i] = dis[i] * ( sum_{e: col=i} hs[row_e] + hs[i] )
"""

import math

import numpy as np

N_NODES = 100000
DIM = 64
N_CORES = 8
P = 128
HROW = 128  # stored h row width (64 data + 64 pad), 256B in bf16


class Cfg:
    def __init__(self, n_nodes=N_NODES, dim=DIM, n_cores=N_CORES,
                 l_run=512, nbg=7, chunk_slices=2):
        self.N = n_nodes
        self.DIM = dim
        self.NC = n_cores
        self.NBLK = math.ceil(n_nodes / (n_cores * P))        # 98
        self.SHARD = self.NBLK * P                            # 12544
        self.SLICE = self.SHARD
        self.NS = n_cores
        self.J = self.NBLK
        self.row_base = np.arange(n_cores + 1) * self.SHARD
        self.H_ROWS = int(self.row_base[-1])                  # 100352
        self.CH_SL = chunk_slices
        self.NCH = math.ceil(self.NS / chunk_slices)          # 4
        self.CROWS = chunk_slices * self.SLICE                # rows per chunk
        assert self.CROWS <= 32767
        self.L_RUN = l_run
        assert l_run % P == 0
        self.TPR = l_run // P                                 # tiles per run
        self.NBG = nbg
        assert self.NBLK % nbg == 0
        self.NGRP = self.NBLK // nbg                          # 14
        self.NRUNS = self.NBLK * self.NCH                     # 392
        self.TOT = self.NRUNS * l_run                         # 200704
        self.NTILES = self.TOT // P                           # 1568
        self.SR = nbg                                         # runs per call
        self.CALL_SLOTS = self.SR * l_run                     # 3584
        self.NCALLS = self.NGRP * self.NCH                    # 56
        self.CW = self.CALL_SLOTS // 16                       # idx cols/call
        self.CT = self.CALL_SLOTS // P                        # slot tiles/call
        self.IDXW = self.TOT // 16                            # 12544


CFG = Cfg()


def rho(cfg, n):
    """node id -> permuted h row (within the full virtual h), vectorized."""
    s = n // cfg.SLICE
    m = n - s * cfg.SLICE
    return cfg.row_base[s] + (m % P) * cfg.J + m // P


# ----------------------------------------------------------------------------
# host preprocessing
# ----------------------------------------------------------------------------

def host_prep(cfg, x, weight, edge_index):
    import ml_dtypes

    x = np.asarray(x, dtype=np.float32)
    weight = np.asarray(weight, dtype=np.float32)
    ei = np.asarray(edge_index)
    row = ei[0].astype(np.int64)
    col = ei[1].astype(np.int64)

    deg = (np.bincount(row, minlength=cfg.N) + 1).astype(np.float32)
    dis = deg ** -0.5
    dis_pad = np.zeros(cfg.H_ROWS, dtype=np.float32)
    dis_pad[:cfg.N] = dis

    # disp[p, s*J + j] = dis[node s*SLICE + j*128 + p]
    nodes = (np.arange(cfg.NS * cfg.SLICE).reshape(cfg.NS, cfg.J, P))
    disp = dis_pad[nodes].transpose(2, 0, 1).reshape(P, cfg.NS * cfg.J)
    disp = np.ascontiguousarray(disp)

    # edge bucketing; calls are chunk-major: call jj = c*NGRP + g
    k = np.minimum(col // cfg.SHARD, cfg.NC - 1)
    blk = (col % cfg.SHARD) // P
    col_local = (col % cfg.SHARD) % P
    g = blk // cfg.NBG
    b_ = blk % cfg.NBG
    c = np.minimum((row // cfg.SLICE) // cfg.CH_SL, cfg.NCH - 1)
    idxrel = rho(cfg, row) - c * cfg.CROWS

    run_in_core = (c * cfg.NGRP + g) * cfg.NBG + b_
    key = k * cfg.NRUNS + run_in_core
    order = np.argsort(key, kind="stable")
    key_s = key[order]
    counts = np.bincount(key_s, minlength=cfg.NC * cfg.NRUNS)
    starts = np.concatenate([[0], np.cumsum(counts)])
    pos = np.arange(key_s.size) - starts[key_s]
    ok = pos < cfg.L_RUN
    slot = run_in_core[order] * cfg.L_RUN + pos
    kk = k[order]

    idx_flat = np.zeros((cfg.NC, cfg.TOT), dtype=np.int16)
    colv = np.full((cfg.NC, cfg.TOT), -1.0, dtype=np.float32)
    o = order[ok]
    idx_flat[kk[ok], slot[ok]] = idxrel[o].astype(np.int16)
    colv[kk[ok], slot[ok]] = col_local[o].astype(np.float32)
    ov = order[~ok]

    colv_p = np.ascontiguousarray(
        colv.reshape(cfg.NC, cfg.NTILES, P).transpose(0, 2, 1))

    idxw = idx_flat.reshape(cfg.NC, cfg.NCALLS, cfg.CW, 16)
    idxw = idxw.transpose(0, 3, 1, 2).reshape(cfg.NC, 16, cfg.IDXW)
    idx_p = np.ascontiguousarray(np.tile(idxw, (1, 8, 1)))

    xt = np.ascontiguousarray(x.T.astype(ml_dtypes.bfloat16))
    if cfg.NS * cfg.SLICE > cfg.N:
        xt = np.concatenate(
            [xt, np.zeros((cfg.DIM, cfg.NS * cfg.SLICE - cfg.N),
                          ml_dtypes.bfloat16)], axis=1)
    wb = weight.astype(ml_dtypes.bfloat16)
    iota4 = np.tile(np.arange(P, dtype=np.float32), (P, cfg.TPR)).copy()

    in_maps = []
    for core in range(cfg.NC):
        n0 = core * cfg.SLICE
        in_maps.append({
            "xt": xt,
            "xself": np.ascontiguousarray(xt[:, n0:n0 + cfg.SLICE]),
            "w": wb,
            "iota4": iota4,
            "disp": disp,
            "disd": np.ascontiguousarray(
                disp[:, core * cfg.J:(core + 1) * cfg.J]),
            "colv": colv_p[core],
            "idx": idx_p[core],
        })

    corr = None
    if ov.size:
        r, cdst = row[ov], col[ov]
        hsrc = x[r] @ weight
        m = hsrc * (dis[r] * dis[cdst])[:, None]
        corr = np.zeros((cfg.N, cfg.DIM), dtype=np.float32)
        np.add.at(corr, cdst, m)
    return in_maps, corr


def unshard(cfg, outs, corr):
    out = np.empty((cfg.N, cfg.DIM), dtype=np.float32)
    for core in range(cfg.NC):
        o = outs[core]["outp"].reshape(P, cfg.NBLK, cfg.DIM)
        o = o.transpose(1, 0, 2).reshape(cfg.SHARD, cfg.DIM)
        nd = min(cfg.SHARD, cfg.N - core * cfg.SHARD)
        out[core * cfg.SHARD:core * cfg.SHARD + nd] = o[:nd]
    if corr is not None:
        out += corr
    return out


# ----------------------------------------------------------------------------
# device program
# ----------------------------------------------------------------------------

_PROG_CACHE = {}

PSB = 8  # h tiles per psum batch in phase 1


def build_program(cfg, reps=1, phases="12", queue_map=None):
    import contextlib

    import concourse.bass as bass
    import concourse.tile as tile
    from concourse import bacc, mybir

    f32 = mybir.dt.float32
    bf16 = mybir.dt.bfloat16
    nc = bacc.Bacc("TRN2", target_bir_lowering=False, debug=False,
                   num_devices=cfg.NC, num_swdge_queues=4)

    NSL = cfg.NS * cfg.SLICE
    xt = nc.dram_tensor("xt", [cfg.DIM, NSL], bf16, kind="ExternalInput")
    xself = nc.dram_tensor("xself", [cfg.DIM, cfg.SLICE], bf16,
                           kind="ExternalInput")
    w = nc.dram_tensor("w", [cfg.DIM, cfg.DIM], bf16, kind="ExternalInput")
    iota4 = nc.dram_tensor("iota4", [P, cfg.TPR * P], f32,
                           kind="ExternalInput")
    disp = nc.dram_tensor("disp", [P, cfg.NS * cfg.J], f32,
                          kind="ExternalInput")
    disd = nc.dram_tensor("disd", [P, cfg.J], f32, kind="ExternalInput")
    colv = nc.dram_tensor("colv", [P, cfg.NTILES], f32, kind="ExternalInput")
    idx = nc.dram_tensor("idx", [P, cfg.IDXW], mybir.dt.int16,
                         kind="ExternalInput")
    outp = nc.dram_tensor("outp", [P, cfg.NBLK * cfg.DIM], f32,
                          kind="ExternalOutput")
    h_ch = [nc.dram_tensor(f"h_ch{c}", [cfg.CROWS, HROW], bf16)
            for c in range(cfg.NCH)]

    nc._gather_insts = []

    def h_pass(sp, pp, src_dram, out_view, dis_sb, dis_off, w_sb, jtot):
        """out_view(j0, jn)[p, j, d] = dis * (src chunk @ W)."""
        for m in range(math.ceil(jtot / PSB)):
            j0 = m * PSB
            jn = min(PSB, jtot - j0)
            xs = sp.tile([cfg.DIM, PSB * P], bf16, tag="xs")
            nc.sync.dma_start(out=xs[:, :jn * P],
                              in_=src_dram[:, j0 * P:(j0 + jn) * P])
            ps = pp.tile([P, PSB * cfg.DIM], f32)
            for j8 in range(jn):
                nc.tensor.matmul(
                    out=ps[:, j8 * cfg.DIM:(j8 + 1) * cfg.DIM],
                    lhsT=xs[:, j8 * P:(j8 + 1) * P],
                    rhs=w_sb[:], start=True, stop=True)
            db = dis_sb[:, dis_off + j0:dis_off + j0 + jn]
            nc.vector.tensor_tensor(
                out=out_view(j0, jn),
                in0=ps[:, :jn * cfg.DIM].rearrange("p (j d) -> p j d", j=jn),
                in1=db.unsqueeze(2).broadcast_to([P, jn, cfg.DIM]),
                op=mybir.AluOpType.mult)

    with tile.TileContext(nc) as tc:
      with (tc.For_i(0, reps, 1) if reps > 1 else contextlib.nullcontext()):
        with tc.tile_pool(name="cst", bufs=1) as cp, \
             tc.tile_pool(name="p1s", bufs=2) as sp, \
             tc.tile_pool(name="p1h", bufs=2) as hp, \
             tc.tile_pool(name="p1p", bufs=4, space="PSUM") as pp, \
             tc.tile_pool(name="p2g", bufs=6) as gp, \
             tc.tile_pool(name="p2sel", bufs=8) as selp, \
             tc.tile_pool(name="p2p", bufs=4, space="PSUM") as pp2:
            w_sb = cp.tile([cfg.DIM, cfg.DIM], bf16)
            nc.sync.dma_start(out=w_sb[:], in_=w[:])
            disd_sb = cp.tile([P, cfg.J], f32)
            nc.sync.dma_start(out=disd_sb[:], in_=disd[:])

            do2 = "2" in phases or "G" in phases
            gather_only = do2 and "2" not in phases

            # ---- phase 1.5 first: self rows = dis * (xself @ W), fp32 ----
            if do2:
                hself = cp.tile([P, cfg.J * cfg.DIM], f32)
                hself3 = hself[:].rearrange("p (j d) -> p j d", j=cfg.J)

                def ovs(j0, jn, hself3=hself3):
                    return hself3[:, j0:j0 + jn, :]
                h_pass(sp, pp, xself, ovs, disd_sb, 0, w_sb, cfg.J)

            # ---- phase 1: h chunks = dis * (x @ W), bf16 padded ----------
            if "1" in phases:
                disp_sb = cp.tile([P, cfg.NS * cfg.J], f32)
                nc.sync.dma_start(out=disp_sb[:], in_=disp[:])
                for s in range(cfg.NS):
                    hs = hp.tile([P, cfg.J * HROW], bf16, tag="hs")
                    hs3 = hs[:].rearrange("p (j d) -> p j d", j=cfg.J)

                    def ov(j0, jn, hs3=hs3):
                        return hs3[:, j0:j0 + jn, 0:cfg.DIM]
                    h_pass(sp, pp, xt[:, s * cfg.SLICE:(s + 1) * cfg.SLICE],
                           ov, disp_sb, s * cfg.J, w_sb, cfg.J)
                    r0 = (s % cfg.CH_SL) * cfg.SLICE
                    dst = h_ch[s // cfg.CH_SL][r0:r0 + cfg.SLICE, :]
                    dst = dst.rearrange("(p j) d -> p (j d)", p=P)
                    nc.sync.dma_start(out=dst, in_=hs[:])

            # ---- phase 2: gather + PE scatter-add, chunk-major -----------
            if do2:
                iota_sb = cp.tile([P, cfg.TPR * P], f32)
                nc.sync.dma_start(out=iota_sb[:], in_=iota4[:])
                colv_sb = cp.tile([P, cfg.NTILES], f32)
                nc.sync.dma_start(out=colv_sb[:], in_=colv[:])
                idx_sb = cp.tile([P, cfg.IDXW], mybir.dt.int16)
                nc.sync.dma_start(out=idx_sb[:], in_=idx[:])
                out_sb = cp.tile([P, cfg.NBLK * cfg.DIM], f32)
                if not gather_only:
                    nc.vector.tensor_copy(out=out_sb[:], in_=hself[:])

                for c in range(cfg.NCH):
                    for g in range(cfg.NGRP):
                        j = c * cfg.NGRP + g
                        gb = gp.tile([P, cfg.CT, HROW], bf16, tag="gb")
                        gi = nc.gpsimd.dma_gather(
                            out_ap=gb[:],
                            in_ap=h_ch[c][:],
                            idxs_ap=idx_sb[:, j * cfg.CW:(j + 1) * cfg.CW],
                            num_idxs=cfg.CALL_SLOTS,
                            num_idxs_reg=cfg.CALL_SLOTS,
                            elem_size=HROW,
                            single_packet=False,
                            queue_num=(queue_map or {}).get(j, 0),
                        )
                        nc._gather_insts.append((j, gi.ins.name))
                        if gather_only:
                            if j == cfg.NCALLS - 1:
                                nc.vector.tensor_copy(
                                    out=out_sb[:, :cfg.DIM],
                                    in_=gb[:, 0, 0:cfg.DIM])
                            continue
                        ps = pp2.tile([P, cfg.NBG * cfg.DIM], f32)
                        for b_ in range(cfg.NBG):
                            T0 = j * cfg.CT + b_ * cfg.TPR
                            sel = selp.tile([P, cfg.TPR * P], bf16, tag="sel")
                            cb = colv_sb[:, T0:T0 + cfg.TPR]
                            nc.vector.tensor_tensor(
                                out=sel[:].rearrange("p (t q) -> p t q",
                                                     t=cfg.TPR),
                                in0=iota_sb[:].rearrange("p (t q) -> p t q",
                                                         t=cfg.TPR),
                                in1=cb.unsqueeze(2).broadcast_to(
                                    [P, cfg.TPR, P]),
                                op=mybir.AluOpType.is_equal)
                            for t in range(cfg.TPR):
                                nc.tensor.matmul(
                                    out=ps[:, b_ * cfg.DIM:(b_ + 1) * cfg.DIM],
                                    lhsT=sel[:, t * P:(t + 1) * P],
                                    rhs=gb[:, b_ * cfg.TPR + t, 0:cfg.DIM],
                                    start=(t == 0),
                                    stop=(t == cfg.TPR - 1),
                                    skip_group_check=True)
                        gr = out_sb[:, g * cfg.NBG * cfg.DIM:
                                    (g + 1) * cfg.NBG * cfg.DIM]
                        nc.vector.tensor_tensor(out=gr, in0=gr, in1=ps[:],
                                                op=mybir.AluOpType.add)
                if not gather_only:
                    nc.vector.tensor_tensor(
                        out=out_sb[:].rearrange("p (b d) -> p b d",
                                                b=cfg.NBLK),
                        in0=out_sb[:].rearrange("p (b d) -> p b d",
                                                b=cfg.NBLK),
                        in1=disd_sb[:].unsqueeze(2).broadcast_to(
                            [P, cfg.NBLK, cfg.DIM]),
                        op=mybir.AluOpType.mult)
                nc.sync.dma_start(out=outp[:], in_=out_sb[:])

    nc.compile()
    return nc


def gather_queue_map(nc):
    """call_j -> queue: DMASW lane % 4; lanes shared with other Pool DMAs
    are pinned to queue 0."""
    from concourse.tile_sem_assignment import PROC_NAME_TO_IDX
    idx2name = {v: k for k, v in PROC_NAME_TO_IDX.items()}
    gather_names = {name for _, name in nc._gather_insts}
    locked = set()
    for name, inst in nc.inst_map.items():
        proc = idx2name.get(getattr(inst, "bass_scheduled_proc", None), "")
        if proc.startswith("DMASW") and name not in gather_names:
            locked.add(proc)
    qm = {}
    for j, name in nc._gather_insts:
        inst = nc.inst_map[name]
        proc = idx2name[inst.bass_scheduled_proc]
        assert proc.startswith("DMASW")
        qm[j] = 0 if proc in locked else int(proc[5:]) % 4
    return qm


def build_with_queues(cfg, reps=1, phases="12"):
    qm = {}
    for _ in range(3):
        nc = build_program(cfg, reps=reps, phases=phases, queue_map=qm)
        qm2 = gather_queue_map(nc)
        if qm2 == qm:
            return nc
        qm = qm2
    return nc


def get_program(cfg, reps=1, phases="12"):
    key = (reps, phases)
    if key not in _PROG_CACHE:
        _PROG_CACHE[key] = build_with_queues(cfg, reps=reps, phases=phases)
    return _PROG_CACHE[key]


# ----------------------------------------------------------------------------
# entry point
# ----------------------------------------------------------------------------

def kernel(x, weight, edge_index):
    from concourse.bass_utils import run_bass_kernel_spmd

    cfg = CFG
    in_maps, corr = host_prep(cfg, x, weight, edge_index)
    nc = get_program(cfg)
    res = run_bass_kernel_spmd(nc, in_maps, list(range(cfg.NC)))
    return unshard(cfg, res.results, corr)


# revision 4
# speedup vs baseline: 1.3269x; 1.1201x over previous
"""GraphGym GeneralConv (GCN-style, add-aggr, symmetric norm) on 8 TRN2 cores.

v3 — v2 plus:
  * phase-1 matmuls and x inputs in bf16 (fp32 PE matmuls are ~8x slower)
  * h_perm split into 4 per-chunk DRAM tensors and gather calls ordered
    chunk-major, so chunk-c gathers only wait for slices 2c/2c+1 — phase 1
    overlaps under the gather pipe
  * per-call PSUM accumulated into an SBUF accumulator by DVE (seeded with
    the self term), final dis[dest] scale as one broadcast multiply
  * the self pass (xself @ W) runs before phase 1 so the seed is ready early

Math:
    deg[i] = 1 + outdeg[i];  dis = deg**-0.5
    hs[r]  = dis[r] * (x[r] @ W)           (stored permuted, bf16, 128-wide)
    out[i] = dis[i] * ( sum_{e: col=i} hs[row_e] + hs[i] )
"""

import math

import numpy as np

N_NODES = 100000
DIM = 64
N_CORES = 8
P = 128
HROW = 128  # stored h row width (64 data + 64 pad), 256B in bf16


class Cfg:
    def __init__(self, n_nodes=N_NODES, dim=DIM, n_cores=N_CORES,
                 l_run=512, nbg=7, chunk_slices=2, gpc=1):
        self.N = n_nodes
        self.DIM = dim
        self.NC = n_cores
        self.NBLK = math.ceil(n_nodes / (n_cores * P))        # 98
        self.SHARD = self.NBLK * P                            # 12544
        self.SLICE = self.SHARD
        self.NS = n_cores
        self.J = self.NBLK
        self.row_base = np.arange(n_cores + 1) * self.SHARD
        self.H_ROWS = int(self.row_base[-1])                  # 100352
        self.CH_SL = chunk_slices
        self.NCH = math.ceil(self.NS / chunk_slices)          # 4
        self.CROWS = chunk_slices * self.SLICE                # rows per chunk
        assert self.CROWS <= 32767
        self.L_RUN = l_run
        assert l_run % P == 0
        self.TPR = l_run // P                                 # tiles per run
        self.NBG = nbg
        assert self.NBLK % nbg == 0
        self.NGRP = self.NBLK // nbg                          # 14
        self.NRUNS = self.NBLK * self.NCH                     # 392
        self.TOT = self.NRUNS * l_run                         # 200704
        self.NTILES = self.TOT // P                           # 1568
        self.GPC = gpc                                        # groups per call
        assert self.NGRP % gpc == 0
        self.SR = nbg * gpc                                   # runs per call
        self.CALL_SLOTS = self.SR * l_run                     # 3584 * gpc
        self.NCALLS = self.NGRP * self.NCH // gpc             # 56 / gpc
        self.CW = self.CALL_SLOTS // 16                       # idx cols/call
        self.CT = self.CALL_SLOTS // P                        # slot tiles/call
        self.GT = self.NBG * self.TPR                         # tiles per group
        self.IDXW = self.TOT // 16                            # 12544


CFG = Cfg()


def rho(cfg, n):
    """node id -> permuted h row (within the full virtual h), vectorized."""
    s = n // cfg.SLICE
    m = n - s * cfg.SLICE
    return cfg.row_base[s] + (m % P) * cfg.J + m // P


# ----------------------------------------------------------------------------
# host preprocessing
# ----------------------------------------------------------------------------

def host_prep(cfg, x, weight, edge_index):
    import ml_dtypes

    x = np.asarray(x, dtype=np.float32)
    weight = np.asarray(weight, dtype=np.float32)
    ei = np.asarray(edge_index)
    row = ei[0].astype(np.int64)
    col = ei[1].astype(np.int64)

    deg = (np.bincount(row, minlength=cfg.N) + 1).astype(np.float32)
    dis = deg ** -0.5
    dis_pad = np.zeros(cfg.H_ROWS, dtype=np.float32)
    dis_pad[:cfg.N] = dis

    # disd[p, s*J + j] = dis[node s*SLICE + j*128 + p] restricted per core
    nodes = (np.arange(cfg.NS * cfg.SLICE).reshape(cfg.NS, cfg.J, P))
    disp = dis_pad[nodes].transpose(2, 0, 1).reshape(P, cfg.NS * cfg.J)
    disp = np.ascontiguousarray(disp)
    xs_pre = x * dis[:, None]  # fold dis[row] into x rows

    # edge bucketing; calls are chunk-major: call jj = c*NGRP + g
    k = np.minimum(col // cfg.SHARD, cfg.NC - 1)
    blk = (col % cfg.SHARD) // P
    col_local = (col % cfg.SHARD) % P
    g = blk // cfg.NBG
    b_ = blk % cfg.NBG
    c = np.minimum((row // cfg.SLICE) // cfg.CH_SL, cfg.NCH - 1)
    idxrel = rho(cfg, row) - c * cfg.CROWS

    run_in_core = (c * cfg.NGRP + g) * cfg.NBG + b_
    key = k * cfg.NRUNS + run_in_core
    order = np.argsort(key, kind="stable")
    key_s = key[order]
    counts = np.bincount(key_s, minlength=cfg.NC * cfg.NRUNS)
    starts = np.concatenate([[0], np.cumsum(counts)])
    pos = np.arange(key_s.size) - starts[key_s]
    ok = pos < cfg.L_RUN
    slot = run_in_core[order] * cfg.L_RUN + pos
    kk = k[order]

    idx_flat = np.zeros((cfg.NC, cfg.TOT), dtype=np.int16)
    colv = np.full((cfg.NC, cfg.TOT), -1.0, dtype=np.float32)
    o = order[ok]
    idx_flat[kk[ok], slot[ok]] = idxrel[o].astype(np.int16)
    colv[kk[ok], slot[ok]] = col_local[o].astype(np.float32)
    ov = order[~ok]

    colv_p = np.ascontiguousarray(
        colv.reshape(cfg.NC, cfg.NTILES, P).transpose(0, 2, 1)
        .astype(ml_dtypes.bfloat16))

    idxw = idx_flat.reshape(cfg.NC, cfg.NCALLS, cfg.CW, 16)
    idxw = idxw.transpose(0, 3, 1, 2).reshape(cfg.NC, 16, cfg.IDXW)
    idx_p = np.ascontiguousarray(np.tile(idxw, (1, 8, 1)))

    xt = np.ascontiguousarray(xs_pre.T.astype(ml_dtypes.bfloat16))
    if cfg.NS * cfg.SLICE > cfg.N:
        xt = np.concatenate(
            [xt, np.zeros((cfg.DIM, cfg.NS * cfg.SLICE - cfg.N),
                          ml_dtypes.bfloat16)], axis=1)
    wb = weight.astype(ml_dtypes.bfloat16)
    iota4 = np.tile(np.arange(P, dtype=ml_dtypes.bfloat16),
                    (P, cfg.TPR)).copy()

    in_maps = []
    for core in range(cfg.NC):
        n0 = core * cfg.SLICE
        in_maps.append({
            "xt": xt,
            "xself": np.ascontiguousarray(xt[:, n0:n0 + cfg.SLICE]),
            "w": wb,
            "iota4": iota4,
            "disd": np.ascontiguousarray(
                disp[:, core * cfg.J:(core + 1) * cfg.J]),
            "colv": colv_p[core],
            "idx": idx_p[core],
        })

    corr = None
    if ov.size:
        r, cdst = row[ov], col[ov]
        hsrc = x[r] @ weight
        m = hsrc * (dis[r] * dis[cdst])[:, None]
        corr = np.zeros((cfg.N, cfg.DIM), dtype=np.float32)
        np.add.at(corr, cdst, m)
    return in_maps, corr


def unshard(cfg, outs, corr):
    out = np.empty((cfg.N, cfg.DIM), dtype=np.float32)
    for core in range(cfg.NC):
        o = outs[core]["outp"].reshape(P, cfg.NBLK, cfg.DIM)
        o = o.transpose(1, 0, 2).reshape(cfg.SHARD, cfg.DIM)
        nd = min(cfg.SHARD, cfg.N - core * cfg.SHARD)
        out[core * cfg.SHARD:core * cfg.SHARD + nd] = o[:nd]
    if corr is not None:
        out += corr
    return out


# ----------------------------------------------------------------------------
# device program
# ----------------------------------------------------------------------------

_PROG_CACHE = {}

PSB = 8  # h tiles per psum batch in phase 1


def build_program(cfg, reps=1, phases="12", queue_map=None):
    import contextlib

    import concourse.bass as bass
    import concourse.tile as tile
    from concourse import bacc, mybir

    f32 = mybir.dt.float32
    bf16 = mybir.dt.bfloat16
    nc = bacc.Bacc("TRN2", target_bir_lowering=False, debug=False,
                   num_devices=cfg.NC, num_swdge_queues=4)

    NSL = cfg.NS * cfg.SLICE
    xt = nc.dram_tensor("xt", [cfg.DIM, NSL], bf16, kind="ExternalInput")
    xself = nc.dram_tensor("xself", [cfg.DIM, cfg.SLICE], bf16,
                           kind="ExternalInput")
    w = nc.dram_tensor("w", [cfg.DIM, cfg.DIM], bf16, kind="ExternalInput")
    iota4 = nc.dram_tensor("iota4", [P, cfg.TPR * P], bf16,
                           kind="ExternalInput")
    disd = nc.dram_tensor("disd", [P, cfg.J], f32, kind="ExternalInput")
    colv = nc.dram_tensor("colv", [P, cfg.NTILES], bf16,
                          kind="ExternalInput")
    idx = nc.dram_tensor("idx", [P, cfg.IDXW], mybir.dt.int16,
                         kind="ExternalInput")
    outp = nc.dram_tensor("outp", [P, cfg.NBLK * cfg.DIM], f32,
                          kind="ExternalOutput")
    h_ch = [nc.dram_tensor(f"h_ch{c}", [cfg.CROWS, HROW], bf16)
            for c in range(cfg.NCH)]

    nc._gather_insts = []

    XB = 32  # x blocks per load chunk (4096 cols)

    def h_pass(sp, pp, src_dram, out_view, w_sb, jtot):
        """out_view(j0, jn)[p, j, d] = (src chunk @ W); x is dis-prescaled."""
        for x0 in range(0, jtot, XB):
            xn = min(XB, jtot - x0)
            xs = sp.tile([cfg.DIM, XB * P], bf16, tag="xs")
            nc.sync.dma_start(out=xs[:, :xn * P],
                              in_=src_dram[:, x0 * P:(x0 + xn) * P])
            for m in range(math.ceil(xn / PSB)):
                j0 = m * PSB
                jn = min(PSB, xn - j0)
                ps = pp.tile([P, PSB * cfg.DIM], f32)
                for j8 in range(jn):
                    nc.tensor.matmul(
                        out=ps[:, j8 * cfg.DIM:(j8 + 1) * cfg.DIM],
                        lhsT=xs[:, (j0 + j8) * P:(j0 + j8 + 1) * P],
                        rhs=w_sb[:], start=True, stop=True)
                nc.scalar.copy(
                    out=out_view(x0 + j0, jn),
                    in_=ps[:, :jn * cfg.DIM].rearrange("p (j d) -> p j d",
                                                       j=jn))

    with tile.TileContext(nc) as tc:
      with (tc.For_i(0, reps, 1) if reps > 1 else contextlib.nullcontext()):
        with tc.tile_pool(name="cst", bufs=1) as cp, \
             tc.tile_pool(name="p1s", bufs=2) as sp, \
             tc.tile_pool(name="p1h", bufs=2) as hp, \
             tc.tile_pool(name="p1p", bufs=4, space="PSUM") as pp, \
             tc.tile_pool(name="p2g", bufs=6) as gp, \
             tc.tile_pool(name="p2sel", bufs=8) as selp, \
             tc.tile_pool(name="p2p", bufs=4, space="PSUM") as pp2:
            w_sb = cp.tile([cfg.DIM, cfg.DIM], bf16)
            nc.sync.dma_start(out=w_sb[:], in_=w[:])
            disd_sb = cp.tile([P, cfg.J], f32)
            nc.sync.dma_start(out=disd_sb[:], in_=disd[:])

            do2 = "2" in phases or "G" in phases
            gather_only = do2 and "2" not in phases

            # ---- phase 1.5 first: self rows = dis * (xself @ W), fp32 ----
            if do2:
                hself = cp.tile([P, cfg.J * cfg.DIM], f32)
                hself3 = hself[:].rearrange("p (j d) -> p j d", j=cfg.J)

                def ovs(j0, jn, hself3=hself3):
                    return hself3[:, j0:j0 + jn, :]
                h_pass(sp, pp, xself, ovs, w_sb, cfg.J)

            # ---- phase 1: h chunks = dis * (x @ W), bf16 padded ----------
            if "1" in phases:
                for s in range(cfg.NS):
                    hs = hp.tile([P, cfg.J * HROW], bf16, tag="hs")
                    hs3 = hs[:].rearrange("p (j d) -> p j d", j=cfg.J)

                    def ov(j0, jn, hs3=hs3):
                        return hs3[:, j0:j0 + jn, 0:cfg.DIM]
                    h_pass(sp, pp, xt[:, s * cfg.SLICE:(s + 1) * cfg.SLICE],
                           ov, w_sb, cfg.J)
                    r0 = (s % cfg.CH_SL) * cfg.SLICE
                    dst = h_ch[s // cfg.CH_SL][r0:r0 + cfg.SLICE, :]
                    dst = dst.rearrange("(p j) d -> p (j d)", p=P)
                    nc.sync.dma_start(out=dst, in_=hs[:])

            # ---- phase 2: gather + PE scatter-add, chunk-major -----------
            if do2:
                iota_sb = cp.tile([P, cfg.TPR * P], bf16)
                nc.sync.dma_start(out=iota_sb[:], in_=iota4[:])
                colv_sb = cp.tile([P, cfg.NTILES], bf16)
                nc.sync.dma_start(out=colv_sb[:], in_=colv[:])
                idx_sb = cp.tile([P, cfg.IDXW], mybir.dt.int16)
                nc.sync.dma_start(out=idx_sb[:], in_=idx[:])
                out_sb = cp.tile([P, cfg.NBLK * cfg.DIM], f32)
                if not gather_only:
                    nc.vector.tensor_copy(out=out_sb[:], in_=hself[:])

                NCC = cfg.NGRP // cfg.GPC   # calls per chunk
                for c in range(cfg.NCH):
                    for gc in range(NCC):
                        j = c * NCC + gc
                        gb = gp.tile([P, cfg.CT, HROW], bf16, tag="gb")
                        gi = nc.gpsimd.dma_gather(
                            out_ap=gb[:],
                            in_ap=h_ch[c][:],
                            idxs_ap=idx_sb[:, j * cfg.CW:(j + 1) * cfg.CW],
                            num_idxs=cfg.CALL_SLOTS,
                            num_idxs_reg=cfg.CALL_SLOTS,
                            elem_size=HROW,
                            single_packet=False,
                            queue_num=(queue_map or {}).get(j, 0),
                        )
                        nc._gather_insts.append((j, gi.ins.name))
                        if gather_only:
                            if j == cfg.NCALLS - 1:
                                nc.vector.tensor_copy(
                                    out=out_sb[:, :cfg.DIM],
                                    in_=gb[:, 0, 0:cfg.DIM])
                            continue
                        for gi_ in range(cfg.GPC):
                            g = gc * cfg.GPC + gi_
                            ps = pp2.tile([P, cfg.NBG * cfg.DIM], f32)
                            for b_ in range(cfg.NBG):
                                T0 = (j * cfg.CT + gi_ * cfg.GT +
                                      b_ * cfg.TPR)
                                sel = selp.tile([P, cfg.TPR * P], bf16,
                                                tag="sel")
                                cb = colv_sb[:, T0:T0 + cfg.TPR]
                                nc.vector.tensor_tensor(
                                    out=sel[:].rearrange("p (t q) -> p t q",
                                                         t=cfg.TPR),
                                    in0=iota_sb[:].rearrange(
                                        "p (t q) -> p t q", t=cfg.TPR),
                                    in1=cb.unsqueeze(2).broadcast_to(
                                        [P, cfg.TPR, P]),
                                    op=mybir.AluOpType.is_equal)
                                for t in range(cfg.TPR):
                                    nc.tensor.matmul(
                                        out=ps[:, b_ * cfg.DIM:
                                               (b_ + 1) * cfg.DIM],
                                        lhsT=sel[:, t * P:(t + 1) * P],
                                        rhs=gb[:, gi_ * cfg.GT +
                                               b_ * cfg.TPR + t, 0:cfg.DIM],
                                        start=(t == 0),
                                        stop=(t == cfg.TPR - 1),
                                        skip_group_check=True)
                            gr = out_sb[:, g * cfg.NBG * cfg.DIM:
                                        (g + 1) * cfg.NBG * cfg.DIM]
                            nc.vector.tensor_tensor(out=gr, in0=gr, in1=ps[:],
                                                    op=mybir.AluOpType.add)
                if not gather_only:
                    nc.vector.tensor_tensor(
                        out=out_sb[:].rearrange("p (b d) -> p b d",
                                                b=cfg.NBLK),
                        in0=out_sb[:].rearrange("p (b d) -> p b d",
                                                b=cfg.NBLK),
                        in1=disd_sb[:].unsqueeze(2).broadcast_to(
                            [P, cfg.NBLK, cfg.DIM]),
                        op=mybir.AluOpType.mult)
                nc.sync.dma_start(out=outp[:], in_=out_sb[:])

    nc.compile()
    return nc


def gather_queue_map(nc):
    """call_j -> queue: DMASW lane % 4; lanes shared with other Pool DMAs
    are pinned to queue 0."""
    from concourse.tile_sem_assignment import PROC_NAME_TO_IDX
    idx2name = {v: k for k, v in PROC_NAME_TO_IDX.items()}
    gather_names = {name for _, name in nc._gather_insts}
    locked = set()
    for name, inst in nc.inst_map.items():
        proc = idx2name.get(getattr(inst, "bass_scheduled_proc", None), "")
        if proc.startswith("DMASW") and name not in gather_names:
            locked.add(proc)
    qm = {}
    for j, name in nc._gather_insts:
        inst = nc.inst_map[name]
        proc = idx2name[inst.bass_scheduled_proc]
        assert proc.startswith("DMASW")
        qm[j] = 0 if proc in locked else int(proc[5:]) % 4
    return qm


def build_with_queues(cfg, reps=1, phases="12"):
    qm = {}
    for _ in range(3):
        nc = build_program(cfg, reps=reps, phases=phases, queue_map=qm)
        qm2 = gather_queue_map(nc)
        if qm2 == qm:
            return nc
        qm = qm2
    return nc


def get_program(cfg, reps=1, phases="12"):
    key = (reps, phases)
    if key not in _PROG_CACHE:
        _PROG_CACHE[key] = build_with_queues(cfg, reps=reps, phases=phases)
    return _PROG_CACHE[key]


# ----------------------------------------------------------------------------
# entry point
# ----------------------------------------------------------------------------

def kernel(x, weight, edge_index):
    from concourse.bass_utils import run_bass_kernel_spmd

    cfg = CFG
    in_maps, corr = host_prep(cfg, x, weight, edge_index)
    nc = get_program(cfg)
    res = run_bass_kernel_spmd(nc, in_maps, list(range(cfg.NC)))
    return unshard(cfg, res.results, corr)
